# revision 52
# baseline (speedup 1.0000x reference)
"""Paged sparse-attention (prefill + paged prefix) Trainium2 kernel.

Sharding: tensor-parallel over KV heads — 8 KV heads across 8 NeuronCores.
Each core handles 1 KV head and its 4 GQA query heads for all 4 sequences.
No collectives needed (heads are independent); host concatenates outputs.

Math: reference = LSE-merge of (causal attn over new tokens) and (non-causal
attn over paged prefix) == single softmax over concatenated [prefix; new]
keys with a causal mask on the new-token block. Scores are small (|s| <~ 6)
so max-subtraction is skipped (exp cannot overflow); the causal mask is a
0/1 multiply on the two diagonal 128-blocks after exp.

All inputs are cast to bf16 on the host (the kernel computed in bf16 anyway,
so numerics are identical) — halves HBM traffic and removes all on-chip
f32->bf16 bounce casts.

Per core, per sequence b, per 128-key chunk j (S^T layout: keys on
partitions, (g, s) query columns folded to nq=1024):
  S^T[j]  = K_chunk_j @ Q'^T        (bf16 matmuls, K^T chunk stationary)
  P^T[j]  = exp(S^T[j] / sqrt(dh))  split per chunk: ScalarE LUT exp on
            cols 0:ACT_COLS, VectorE piecewise-linear exp in the bf16-bit
            domain on the rest — so per-chunk exp latency stays below the
            per-chunk PE time.
  O[m]   += P^T[j][:, m-chunk].T @ [V_j | 1]  (ones col => softmax denom,
            all 8 m accumulators packed in one 4-bank PSUM tile)
  out[m]  = O[m][:, :128] / O[m][:, 128]

Sequence 0's K and V land in SEPARATE SBUF tiles per DMA piece: the
strided gather APs defeat precise region overlap tracking, so a single
tile makes every consumer wait for the WHOLE gather; separate tiles give
the first chunks their data ~7us earlier in the prologue.
"""

import numpy as np
import ml_dtypes

from concourse import bacc
import concourse.mybir as mybir
import concourse.tile as tile
from concourse.tile_rust import add_dep_helper
from concourse.bass_utils import run_bass_kernel_spmd

# Problem shape (hardcoded per harness contract)
HQ, HKV, DH, PAGE = 32, 8, 128, 16
B, S, PREFIX = 4, 256, 2048
N = B * S                      # 1024 new tokens
NSLOTS = 16384
G = HQ // HKV                  # 4 query heads per kv head
NQ = G * S                     # 1024 query columns per sequence per core
L = PREFIX + S                 # 2304 keys per sequence
JCH = L // 128                 # 18 key chunks of 128
JPRE = PREFIX // 128           # 16 prefix chunks
MCH = NQ // 128                # 8 query chunks of 128
SCALE = DH ** -0.5
NCORES = 8

# per-chunk exp split: ScalarE (ACT) LUT exp on cols [0, ACT_COLS), VectorE
# (DVE) piecewise-linear bf16-bit-domain exp on the rest.
ACT_COLS = 768
FEXP_A = float(SCALE * 128.0 / np.log(2.0))
FEXP_B = float(127.0 * 128.0 - 366393.0 / 65536.0)

F32 = mybir.dt.float32
BF16 = mybir.dt.bfloat16

# K piece cuts (key positions) and V piece cuts (chunk indices)
KCUTS = [0, 256, 1152, L]
VPARTS = [(0, 2), (2, 8), (8, JCH)]   # chunk ranges per V tile (b=0)


def _runs(idx):
    """Coalesce a 1-D int array into (start_pos, start_val, length) runs of
    consecutive values."""
    idx = np.asarray(idx)
    out = []
    st = 0
    for i in range(1, len(idx) + 1):
        if i == len(idx) or idx[i] != idx[i - 1] + 1:
            out.append((st, int(idx[st]), i - st))
            st = i
    return out


def build_bass(slot_idx):
    """slot_idx: [B, PREFIX] int array of gathered cache slots per sequence.
    The gather structure (DMA descriptors) is specialized to these values;
    it is identical across cores (page metadata is replicated)."""
    nc = bacc.Bacc(trn_type="TRN2")

    qT = nc.dram_tensor("qT", [DH, B * NQ], BF16, kind="ExternalInput")
    kTc = nc.dram_tensor("kTc", [DH, NSLOTS], BF16, kind="ExternalInput")
    kTn = nc.dram_tensor("kTn", [DH, N], BF16, kind="ExternalInput")
    # V arrives pre-transposed from the host as [p, chunk, d] (p = slot %
    # 128): a chunk-aligned gather is then a contiguous per-partition slice
    # (128 big descriptors) instead of one 256B descriptor per slot row.
    vc = nc.dram_tensor("vc", [128, NSLOTS // 128, DH], BF16, kind="ExternalInput")
    vn = nc.dram_tensor("vn", [128, N // 128, DH], BF16, kind="ExternalInput")
    maskd = nc.dram_tensor("maskd", [128, 128], BF16, kind="ExternalInput")
    # output rows are (b, half, partition) with 4 m-slots x DH contiguous per
    # row: each store is 128 contiguous 2KB descriptors instead of 512
    # scattered 512B ones; the host reorders to the logical layout.
    out = nc.dram_tensor("out", [B * 2 * 128, 4 * DH], F32, kind="ExternalOutput")

    with tile.TileContext(nc) as tc:
        with (
            tc.tile_pool(name="singles", bufs=1) as singles,
            tc.tile_pool(name="kv", bufs=2) as kv,
            tc.tile_pool(name="pp", bufs=2) as pp,
            tc.tile_pool(name="outp", bufs=4) as outp,
            tc.tile_pool(name="small", bufs=8) as small,
            tc.tile_pool(name="ps_s", bufs=2, space="PSUM") as ps_s,
            tc.tile_pool(name="ps_o", bufs=1, space="PSUM") as ps_o,
        ):
            mask_sb = singles.tile([128, 128], BF16)

            # PE_HAM clock-gate warmup while the prologue DMAs land.
            warm = singles.tile([128, 512], BF16)
            nc.vector.memset(warm[:], 0.0)
            for _ in range(14):
                pw = ps_s.tile([128, NQ], F32, tag="ps")
                nc.tensor.matmul(
                    pw[:, :512],
                    lhsT=warm[:, :128],
                    rhs=warm[:],
                    start=True,
                    stop=True,
                )

            def prep_v0():
                """Sequence 0's V: the first piece (chunks 0-1) rides the SP
                ring FIRST (it drains before Q/K and unblocks PV(0)); the
                rest rides the ACT ring behind the mask load. Each piece is
                its own tile so coarse DMA dep tracking can't couple early
                PV chunks to the whole gather."""
                slots = slot_idx[0]
                base = int(slots[0])  # contiguous run for this input
                vtiles = []
                engs = [nc.sync, nc.scalar, nc.scalar]
                C0 = base // 128
                for pi, (c0, c1) in enumerate(VPARTS):
                    nch = c1 - c0
                    vt = kv.tile([128, nch, DH + 1], BF16, tag=f"vaug0_{pi}")
                    if c1 <= JPRE:
                        engs[pi].dma_start(
                            vt[:, :, :DH], vc[:, C0 + c0 : C0 + c1, :]
                        )
                    else:
                        engs[pi].dma_start(
                            vt[:, : JPRE - c0, :DH],
                            vc[:, C0 + c0 : C0 + JPRE, :],
                        )
                        engs[pi].dma_start(
                            vt[:, JPRE - c0 :, :DH], vn[:, 0 : S // 128, :]
                        )
                    nc.gpsimd.memset(vt[:, :, DH : DH + 1], 1.0)
                    vtiles.append((c0, vt))
                return vtiles

            def prep_v(b):
                """V gather for b>0 (prefetched a sequence ahead, so coarse
                deps are harmless): one tile, pieces on the SP ring."""
                slots = slot_idx[b]
                vaug = kv.tile([128, JCH, DH + 1], BF16, tag="vaug")
                for dst, src, ln in _runs(slots):
                    while ln > 0:
                        if dst % 128 == 0 and src % 128 == 0 and ln >= 128:
                            nch = ln // 128
                            nc.sync.dma_start(
                                vaug[:, dst // 128 : dst // 128 + nch, :DH],
                                vc[:, src // 128 : src // 128 + nch, :],
                            )
                            adv = nch * 128
                        else:
                            # slow fallback: one slot row at a time from the
                            # transposed layout
                            adv = 1
                            nc.sync.dma_start(
                                vaug[dst % 128, dst // 128, :DH],
                                vc[src % 128, src // 128, :],
                            )
                        dst += adv
                        src += adv
                        ln -= adv
                nc.sync.dma_start(
                    vaug[:, JPRE : JPRE + S // 128, :DH],
                    vn[:, b * (S // 128) : (b + 1) * (S // 128), :],
                )
                nc.gpsimd.memset(vaug[:, :, DH : DH + 1], 1.0)
                return [(0, vaug)]

            def prep_qk(b):
                """Q/K DMAs for sequence b. For b=0 each K piece is its own
                tile (see module docstring); Q rides SP after V piece 0."""
                slots = slot_idx[b]
                qT_sb = kv.tile([DH, NQ], BF16, tag="qT_sb")
                nc.sync.dma_start(qT_sb[:], qT[:, b * NQ : (b + 1) * NQ])

                kdmas = [[] for _ in range(len(KCUTS) - 1)]
                if b == 0 and seq0_contig:
                    base = int(slots[0])
                    ktiles = []
                    for ci in range(len(KCUTS) - 1):
                        a, z = KCUTS[ci], KCUTS[ci + 1]
                        kt = kv.tile([128, z - a], BF16, tag=f"kT0_{ci}")
                        if z <= PREFIX:
                            d = nc.sync.dma_start(
                                kt[:], kTc[:, base + a : base + z]
                            )
                            kdmas[ci].append(d)
                        else:
                            d = nc.sync.dma_start(
                                kt[:, : PREFIX - a],
                                kTc[:, base + a : base + PREFIX],
                            )
                            kdmas[ci].append(d)
                            d = nc.sync.dma_start(
                                kt[:, PREFIX - a :], kTn[:, 0:S]
                            )
                            kdmas[ci].append(d)
                        ktiles.append((a, kt))
                else:
                    kT = kv.tile([128, L], BF16, tag="kT")
                    for dst, src, ln in _runs(slots):
                        lo, hi = dst, dst + ln
                        for ci in range(len(KCUTS) - 1):
                            a = max(lo, KCUTS[ci])
                            z = min(hi, KCUTS[ci + 1])
                            if z > a:
                                d = nc.sync.dma_start(
                                    kT[:, a:z],
                                    kTc[:, src + a - dst : src + z - dst],
                                )
                                kdmas[ci].append(d)
                    d = nc.sync.dma_start(
                        kT[:, PREFIX:L], kTn[:, b * S : (b + 1) * S]
                    )
                    kdmas[-1].append(d)
                    ktiles = [(0, kT)]
                return qT_sb, ktiles, kdmas

            def kchunk(ktiles, j):
                """lhsT slice for key chunk j from the piece tiles."""
                for a, kt in reversed(ktiles):
                    if j * 128 >= a:
                        return kt[:, j * 128 - a : (j + 1) * 128 - a]
                raise AssertionError

            def vchunk(vtiles, j):
                for c0, vt in reversed(vtiles):
                    if j >= c0:
                        return vt[:, j - c0, :]
                raise AssertionError

            # Sequence 0 prologue: V piece 0 first on SP, then Q/K on SP,
            # mask + remaining V pieces on the ACT ring. The piece-tile fast
            # path assumes seq 0's slots are one contiguous run; fall back
            # to the general gather otherwise.
            s0 = slot_idx[0]
            seq0_contig = bool(np.array_equal(s0, np.arange(s0[0], s0[0] + PREFIX)))
            vtiles0 = prep_v0() if seq0_contig else prep_v(0)
            nc.scalar.dma_start(mask_sb[:], maskd[:, :])
            qk0 = prep_qk(0)
            preps = {0: (*qk0, vtiles0)}

            exp_chain = []  # per chunk: tuple of pT-producing instrs
            for b in range(B):
                qT_sb, ktiles, kdmas, vtiles = preps.pop(b)

                # nop absorbers: collapse each K-piece's DMA waits into a PE
                # nop so no LDWEIGHTS carries a DMA sem wait (a wait on the
                # fused LDW blocks the HW weight-prefetch reorder).
                kdma_nopped = [False] * len(kdmas)

                def absorb_kdmas(ci):
                    if not kdma_nopped[ci]:
                        kdma_nopped[ci] = True
                        knop = nc.tensor.nop(nofuse=True)
                        for d in kdmas[ci]:
                            add_dep_helper(
                                knop.ins, d.ins, sync=True,
                                reason="absorb K DMA wait off LDWEIGHTS",
                            )

                pT = pp.tile([128, JCH, NQ], BF16, tag="pT")
                po8 = ps_o.tile([128, MCH, 256], F32, tag="po8")
                # the two new-token chunks (16: half PE work after masking,
                # 17: half scores + half PV) are spread apart so the PE
                # doesn't run dry behind them while the exp engines catch up
                j_order = (
                    list(range(8)) + [JPRE, 8, 9, 10, JPRE + 1] + list(range(11, JPRE))
                )
                for jpos, j in enumerate(j_order):
                    if jpos == 14 and b + 1 < B:
                        qk = prep_qk(b + 1)
                        preps[b + 1] = (*qk, prep_v(b + 1))
                    for ci in range(len(KCUTS) - 1):
                        if KCUTS[ci] <= j * 128 < KCUTS[ci + 1]:
                            absorb_kdmas(ci)
                    ps = ps_s.tile([128, NQ], F32, tag="ps")
                    if len(exp_chain) >= 2:
                        # Absorb the ps-slot WAR wait into a nop so the score
                        # matmul's fused LDWEIGHTS is wait-free.
                        wnop = nc.tensor.nop(nofuse=True)
                        for e in exp_chain[-2]:
                            add_dep_helper(
                                wnop.ins, e.ins, sync=True,
                                reason="absorb ps-slot wait off LDWEIGHTS",
                            )
                    if j == JPRE + 1:
                        # the even-m half (s < 128) is fully masked for this
                        # key block and its PV matmuls are skipped: compute
                        # scores/exp/mask for the odd-m columns only
                        qodd = qT_sb.rearrange(
                            "p (g h q) -> p g h q", g=4, h=2
                        )[:, :, 1, :]
                        nc.tensor.matmul(
                            ps[:, :512],
                            lhsT=kchunk(ktiles, j),
                            rhs=qodd,
                            start=True,
                            stop=True,
                        )
                        podd = pT[:, j, :].rearrange(
                            "p (g h q) -> p g h q", g=4, h=2
                        )[:, :, 1, :]
                        exp_chain.append((nc.scalar.activation(
                            out=podd,
                            in_=ps[:, :512],
                            func=mybir.ActivationFunctionType.Exp,
                            scale=SCALE,
                        ),))
                        nc.vector.tensor_tensor(
                            podd,
                            podd,
                            mask_sb[:, None, :].to_broadcast((128, 4, 128)),
                            mybir.AluOpType.mult,
                        )
                    else:
                        for h2 in range(2):
                            nc.tensor.matmul(
                                ps[:, h2 * 512 : (h2 + 1) * 512],
                                lhsT=kchunk(ktiles, j),
                                rhs=qT_sb[:, h2 * 512 : (h2 + 1) * 512],
                                start=True,
                                stop=True,
                            )
                        e_act = nc.scalar.activation(
                            out=pT[:, j, :ACT_COLS],
                            in_=ps[:, :ACT_COLS],
                            func=mybir.ActivationFunctionType.Exp,
                            scale=SCALE,
                        )
                        e_dve = nc.vector.tensor_scalar(
                            pT[:, j, ACT_COLS:].bitcast(mybir.dt.int16),
                            ps[:, ACT_COLS:],
                            FEXP_A,
                            FEXP_B,
                            mybir.AluOpType.mult,
                            mybir.AluOpType.add,
                        )
                        exp_chain.append((e_act, e_dve))
                        if j == JPRE:
                            # only the diagonal 128-blocks need masking: the
                            # even m-chunks (s < 128) for key block 0
                            tri = pT[:, j, :].rearrange(
                                "p (g h q) -> p g h q", g=4, h=2
                            )[:, :, 0, :]
                            nc.vector.tensor_tensor(
                                tri[:],
                                tri[:],
                                mask_sb[:, None, :].to_broadcast((128, 4, 128)),
                                mybir.AluOpType.mult,
                            )
                    # Two m-slots share each PSUM bank; start=True clears
                    # has_written for the WHOLE bank, so only the even m
                    # (bank-first) may use it. The odd m's first matmul
                    # relies on the bank-wide clear (bit unset => overwrite)
                    # and is order-pinned behind the even one.
                    prev_mm = None
                    for m in range(MCH):
                        if j == JCH - 1 and m % 2 == 0:
                            # keys 128..255 of the new block are masked for
                            # every query in an even m-chunk (s < 128): the
                            # whole P^T block is zero -- skip the matmul.
                            continue
                        mm = nc.tensor.matmul(
                            po8[:, m, : DH + 1],
                            lhsT=pT[:, j, m * 128 : (m + 1) * 128],
                            rhs=vchunk(vtiles, j),
                            start=(jpos == 0 and m % 2 == 0),
                            stop=(jpos == JCH - 1),
                            skip_group_check=True,
                        )
                        if jpos == 0:
                            if m % 2 == 1 and prev_mm is not None:
                                add_dep_helper(
                                    mm.ins, prev_mm.ins, sync=False,
                                    reason="has_written bank clear order",
                                )
                            prev_mm = mm

                # ---- normalize: o = po8[:, :, :128] / po8[:, :, 128],
                # in halves so the first store overlaps the second divide ----
                osb_b = outp.tile([128, MCH, DH], F32, tag="osb")
                for hv in range(2):
                    ms = slice(hv * 4, hv * 4 + 4)
                    dinv4 = small.tile([128, 4, 1], F32, tag="dinv4")
                    nc.vector.reciprocal(dinv4[:], po8[:, ms, DH : DH + 1])
                    nc.vector.tensor_tensor(
                        osb_b[:, ms, :],
                        po8[:, ms, :DH],
                        dinv4.to_broadcast([128, 4, DH]),
                        mybir.AluOpType.mult,
                    )
                    r0 = (b * 2 + hv) * 128
                    nc.sync.dma_start(
                        out[r0 : r0 + 128, :].rearrange(
                            "p (m d) -> p m d", m=4
                        ),
                        osb_b[:, ms, :],
                    )
    nc.finalize()
    return nc


def _prepare(q, k, v, k_cache, v_cache, slot_mapping, block_table):
    """Host-side shard prep. Applies the KV-cache scatter (store_kvcache) on
    host copies, then builds per-core head-sharded bf16 arrays."""
    q = np.asarray(q, np.float32)
    k = np.asarray(k, np.float32)
    v = np.asarray(v, np.float32)
    k_cache = np.array(k_cache, np.float32)
    v_cache = np.array(v_cache, np.float32)
    slot_mapping = np.asarray(slot_mapping, np.int64)
    block_table = np.asarray(block_table, np.int64)

    k_cache[slot_mapping] = k
    v_cache[slot_mapping] = v

    slot_idx = (
        block_table[:, :, None] * PAGE + np.arange(PAGE, dtype=np.int64)
    ).reshape(B, PREFIX)

    # the causal mask reduces to ONE lower-triangular [128,128] block: both
    # new-token key chunks mask only their diagonal 128-block, and the
    # triangle is identical for every GQA head and both chunks
    mask = np.triu(np.ones((128, 128))).astype(ml_dtypes.bfloat16)

    bf = ml_dtypes.bfloat16
    in_maps = []
    for h in range(NCORES):
        qh = q[:, h * G * DH : (h + 1) * G * DH]  # [N, 512]
        qT = np.ascontiguousarray(
            qh.reshape(B, S, G, DH).transpose(3, 0, 2, 1).reshape(DH, B * NQ)
        ).astype(bf)
        kTc = np.ascontiguousarray(k_cache[:, h * DH : (h + 1) * DH].T).astype(bf)
        kTn = np.ascontiguousarray(k[:, h * DH : (h + 1) * DH].T).astype(bf)
        vch = np.ascontiguousarray(
            v_cache[:, h * DH : (h + 1) * DH]
            .reshape(NSLOTS // 128, 128, DH)
            .transpose(1, 0, 2)
        ).astype(bf)
        vnh = np.ascontiguousarray(
            v[:, h * DH : (h + 1) * DH]
            .reshape(N // 128, 128, DH)
            .transpose(1, 0, 2)
        ).astype(bf)
        in_maps.append(
            dict(qT=qT, kTc=kTc, kTn=kTn, vc=vch, vn=vnh, maskd=mask)
        )
    return in_maps, slot_idx


def _assemble(results):
    """results: per-core dicts with 'out' [B*2*128, 4*DH], rows (b, hv, qp),
    cols (mi, d) where m = hv*4 + mi = g*2 + s_half. Returns [N, HQ*DH]."""
    full = np.empty((N, HQ * DH), np.float32)
    for h, res in enumerate(results):
        o = res["out"].reshape(B, 2, 128, 2, 2, DH)  # (b, hv, qp, g2, sh, d)
        # g = hv*2 + g2, token s = sh*128 + qp
        oc = o.transpose(0, 4, 2, 1, 3, 5).reshape(N, G * DH)  # (b,sh,qp)(g,d)
        full[:, h * G * DH : (h + 1) * G * DH] = oc
    return full


def _ensure_ntff_hook():
    """The image's `antenv` stub lacks `axon_hooks`; register the same
    ctypes-based NTFF profile hook trn_agent_boot would have installed so
    trace=True / BASS_TRACE=1 profiling works."""
    try:
        import antenv.axon_hooks  # noqa: F401
        return
    except ImportError:
        pass
    import sys
    import types

    mod = types.ModuleType("antenv.axon_hooks")
    mod._hook = None
    mod.set_axon_ntff_profile_hook = lambda h: setattr(mod, "_hook", h)
    mod.get_axon_ntff_profile_hook = lambda: mod._hook
    sys.modules["antenv.axon_hooks"] = mod
    import antenv

    antenv.axon_hooks = mod
    try:
        from trn_agent_boot.trn_boot import _ntff_profile_via_ctypes

        mod._hook = _ntff_profile_via_ctypes("/opt/axon/libaxon_pjrt.so")
    except Exception:
        mod._hook = None


def run(trace=False, **inputs):
    _ensure_ntff_hook()
    in_maps, slot_idx = _prepare(**inputs)
    nc = build_bass(slot_idx)
    res = run_bass_kernel_spmd(
        nc, in_maps, core_ids=list(range(NCORES)), trace=trace
    )
    return _assemble(res.results), res


def kernel(**inputs) -> np.ndarray:
    out, _ = run(trace=False, **inputs)
    return out
